# revision 1
# baseline (speedup 1.0000x reference)
"""CAM-module kernel for Trainium2, 8 NeuronCores, data-parallel over batch.

Per batch b (B=16, C=512, N=H*W=4096), with Q_b = x[b] reshaped (N, C):
    E_b   = Q_b^T Q_b                      (C x C gram, bf16 matmuls)
    mx[d] = max_c E_0[c, d]                (batch 0 ONLY; E symmetric ->
                                            row-max of E_0 works)
    A_b   = softmax(mx - E_b, axis=-1)     (one-hot-ish in practice)
    out_b = gamma * (A_b @ Q_b^T) + x[b]

Sharding: core i handles batches (i, i+8); every core redundantly computes
E_0 (shared x0 input) to obtain mx without collectives.

Host-side prep: x is recast to bf16 in two layouts -- Q (N x C) for the
gram, and Q^T = x_mat (C x N) for the second matmul.  The resident Q^T
tiles double as the epilogue "+x" operand (x_mat layout == output layout).

Layout trick: Q bf16 is loaded as stride-8 row-interleaved tiles
qs16[g][p, k, f] = Q[1024*g + 8*p + k, f]; each [128, 8*512] tile is one
contiguous 2 MB DMA, and slices [:, k, :] are mm1 contraction chunks (sum
order over n is irrelevant).
"""

import os

import numpy as np
import ml_dtypes

B, C, HW = 16, 512, 64 * 64
NCORES = 8
BPC = 2   # batches per core
KC = 8    # n-chunk count (512-wide chunks of HW)
G = 4     # 128-row chunk count (of C, and of the strided n decomposition)
NCH = 32  # mm1 contraction chunks (of 128)

# X32: epilogue adds x in fp32 (extra 16MB/core of loads); otherwise the
# resident bf16 Q tiles provide x (rel err ~1.7e-3 vs ~4e-4).
X32 = os.environ.get("CAM_X32", "1") == "1"

_cache = {}


def _build_nc():
    import concourse.tile as tile
    from concourse import bacc, mybir
    from concourse.masks import make_identity

    f32 = mybir.dt.float32
    bf16 = mybir.dt.bfloat16
    AluOp = mybir.AluOpType
    ActFn = mybir.ActivationFunctionType

    nc = bacc.Bacc("TRN2", target_bir_lowering=False, debug=False,
                   num_devices=NCORES)

    xq16 = nc.dram_tensor("xq16", [BPC, HW, C], bf16, kind="ExternalInput")
    x0q16 = nc.dram_tensor("x0q16", [HW, C], bf16, kind="ExternalInput")
    # host-pretransposed: qt16[j] = Q_j^T  (C x HW)
    qt16 = nc.dram_tensor("qt16", [BPC, C, HW], bf16, kind="ExternalInput")
    gamma = nc.dram_tensor("gamma", [1, 1], f32, kind="ExternalInput")
    out = nc.dram_tensor("out", [BPC, C, HW], f32, kind="ExternalOutput")
    xq32 = None
    if X32:
        xq32 = nc.dram_tensor("xq32", [BPC, HW, C], f32, kind="ExternalInput")

    with tile.TileContext(nc) as tc:
        with (
            tc.tile_pool(name="consts", bufs=1) as consts,
            tc.tile_pool(name="qs", bufs=5) as qsp,       # [128,8,512] bf16 8KB
            tc.tile_pool(name="qt", bufs=5) as qtp,       # [128,8,512] bf16 8KB
            tc.tile_pool(name="x32", bufs=5 if X32 else 1) as x32p,
            tc.tile_pool(name="pp", bufs=6) as ppp,       # P bf16 1KB
            tc.tile_pool(name="pt", bufs=18) as ptp,      # PT bf16 256B
            tc.tile_pool(name="e2", bufs=4) as e2p,       # f32 2KB
            tc.tile_pool(name="res", bufs=6) as resp,     # f32 2KB
            tc.tile_pool(name="small", bufs=5) as smallp,
            tc.tile_pool(name="grp", bufs=8) as grp,
            tc.tile_pool(name="eps", bufs=4, space="PSUM") as epsp,
            tc.tile_pool(name="ops", bufs=4, space="PSUM") as opsp,
        ):
            # ---- constants
            ident = consts.tile([128, 128], f32, name="ident")
            make_identity(nc, ident[:])
            ident_bf = consts.tile([128, 128], bf16, name="ident_bf")
            make_identity(nc, ident_bf[:])
            ones1 = consts.tile([1, 128], f32, name="ones1")
            nc.vector.memset(ones1[:], 1.0)
            gb = consts.tile([128, 1], f32, name="gb")
            nc.gpsimd.dma_start(out=gb[:], in_=gamma.ap().to_broadcast([128, 1]))
            mxb = consts.tile([128, C], f32, name="mxb")

            # ---- phase 0: mx from x0 (E0 = Q0^T Q0; mx = row-max by symmetry)
            x0r = x0q16.ap().rearrange("(g p k) c -> g p k c", k=KC, p=128)
            e0 = [epsp.tile([128, C], f32, name=f"e0_{c0}", tag="e")
                  for c0 in range(G)]
            q0 = []
            for g in range(G):
                t_ = qsp.tile([128, KC, C], bf16, name=f"q0_{g}", tag="qs")
                nc.sync.dma_start(out=t_[:], in_=x0r[g])
                q0.append(t_)
            ci = 0
            for g in range(G):
                for k in range(KC):
                    qk = q0[g][:, k, :]
                    for c0 in range(G):
                        nc.tensor.matmul(
                            e0[c0][:],
                            lhsT=qk[:, c0 * 128:(c0 + 1) * 128],
                            rhs=qk,
                            start=(ci == 0),
                            stop=(ci == NCH - 1),
                        )
                    ci += 1
            mxv = [smallp.tile([128, 1], f32, name=f"mxv_{c0}", tag="mxv")
                   for c0 in range(G)]
            for c0 in range(G):
                nc.vector.reduce_max(out=mxv[c0][:], in_=e0[c0][:],
                                     axis=mybir.AxisListType.X)
            mxrow_ps = opsp.tile([1, C], f32, name="mxrow_ps", tag="o")
            for c0 in range(G):
                nc.tensor.matmul(
                    mxrow_ps[:, c0 * 128:(c0 + 1) * 128],
                    lhsT=mxv[c0][:],
                    rhs=ident[:],
                    start=True,
                    stop=True,
                )
            mxrow = smallp.tile([1, C], f32, name="mxrow", tag="mxrow")
            nc.vector.tensor_copy(mxrow[:], mxrow_ps[:])
            mxb_ps = opsp.tile([128, C], f32, name="mxb_ps", tag="o")
            nc.tensor.matmul(mxb_ps[:], lhsT=ones1[:], rhs=mxrow[:],
                             start=True, stop=True)
            nc.vector.tensor_copy(mxb[:], mxb_ps[:])

            # ---- per-batch pipeline
            for b in range(BPC):
                xq_b = xq16.ap()[b].rearrange("(g p k) c -> g p k c",
                                              k=KC, p=128)
                qt_b = qt16.ap()[b].rearrange("(d p) (k n) -> d p k n",
                                              p=128, n=512)
                out_b = out.ap()[b]

                # Q bf16 (stride-8 interleaved), one 2MB DMA per g
                qs = []
                for g in range(G):
                    t_ = qsp.tile([128, KC, C], bf16, name=f"qs{b}_{g}",
                                  tag="qs")
                    nc.sync.dma_start(out=t_[:], in_=xq_b[g])
                    qs.append(t_)

                # fp32 x for the epilogue (same strided interleave ->
                # x32s[g][:, k, :] = x_mat[128g.. , 512k..])
                x32s = []
                if X32:
                    xq32_b = xq32.ap()[b].rearrange(
                        "(g p k) c -> g p k c", k=KC, p=128)
                    for g in range(G):
                        t_ = x32p.tile([128, KC, C], f32, name=f"x32{b}_{g}",
                                       tag="x32")
                        nc.sync.dma_start(out=t_[:], in_=xq32_b[g])
                        x32s.append(t_)

                # Q^T bf16 tiles (== x_mat layout; also the epilogue +x operand)
                QT = []
                for d0 in range(G):
                    t_ = qtp.tile([128, KC, 512], bf16, name=f"qt{b}_{d0}",
                                  tag="qt")
                    nc.scalar.dma_start(out=t_[:], in_=qt_b[d0])
                    QT.append(t_)

                # mm1: E = Q^T Q (bf16, FWL), accumulate over 32 chunks
                e = [epsp.tile([128, C], f32, name=f"e{b}_{c0}", tag="e")
                     for c0 in range(G)]
                ci = 0
                for g in range(G):
                    for k in range(KC):
                        qk = qs[g][:, k, :]
                        for c0 in range(G):
                            nc.tensor.matmul(
                                e[c0][:],
                                lhsT=qk[:, c0 * 128:(c0 + 1) * 128],
                                rhs=qk,
                                start=(ci == 0),
                                stop=(ci == NCH - 1),
                            )
                        ci += 1

                # softmax (unnormalized): e2 = E - mx ; m2 = min(e2) ;
                # P = exp(-e2 + m2) ; Z = rowsum(P) ; gR = gamma/Z
                P = []
                gR = []
                for c0 in range(G):
                    e2 = e2p.tile([128, C], f32, name=f"e2{b}_{c0}", tag="e2")
                    m2 = smallp.tile([128, 1], f32, name=f"m2{b}_{c0}",
                                     tag="m2")
                    nc.vector.tensor_sub(e2[:], e[c0][:], mxb[:])
                    nc.vector.tensor_reduce(
                        out=m2[:], in_=e2[:], axis=mybir.AxisListType.X,
                        op=AluOp.min,
                    )
                    p_ = ppp.tile([128, C], bf16, name=f"p{b}_{c0}", tag="p")
                    z = smallp.tile([128, 1], f32, name=f"z{b}_{c0}", tag="z")
                    nc.scalar.activation(
                        out=p_[:],
                        in_=e2[:],
                        func=ActFn.Exp,
                        bias=m2[:],
                        scale=-1.0,
                        accum_out=z[:],
                    )
                    r_ = smallp.tile([128, 1], f32, name=f"r{b}_{c0}", tag="r")
                    nc.vector.reciprocal(r_[:], z[:])
                    gr = grp.tile([128, 1], f32, name=f"gr{b}_{c0}",
                                  tag="gr")
                    nc.vector.tensor_mul(gr[:], r_[:], gb[:])
                    P.append(p_)
                    gR.append(gr)

                # PT = P^T via TensorE transpose (DMA transpose corrupts
                # under concurrent load)
                PT = {}
                for d0 in range(G):
                    for c0 in range(G):
                        pt_ps = opsp.tile([128, 128], bf16,
                                          name=f"ptp{b}_{d0}_{c0}", tag="o")
                        nc.tensor.transpose(
                            pt_ps[:],
                            P[c0][:, d0 * 128:(d0 + 1) * 128],
                            ident_bf[:],
                        )
                        t_ = ptp.tile([128, 128], bf16,
                                      name=f"pt{b}_{d0}_{c0}", tag="pt")
                        nc.scalar.copy(t_[:], pt_ps[:])
                        PT[(d0, c0)] = t_

                # mm2 + fused epilogue: out = (P@Q^T) * (gamma/Z) + x
                for n0 in range(KC):
                    for c0 in range(G):
                        o_ps = opsp.tile([128, 512], f32,
                                         name=f"o{b}_{n0}_{c0}", tag="o")
                        for d0 in range(G):
                            nc.tensor.matmul(
                                o_ps[:],
                                lhsT=PT[(d0, c0)][:],
                                rhs=QT[d0][:, n0, :],
                                start=(d0 == 0),
                                stop=(d0 == G - 1),
                            )
                        x_op = (x32s[c0][:, n0, :] if X32
                                else qs[c0][:, n0, :])
                        res = resp.tile([128, 512], f32,
                                        name=f"res{b}_{n0}_{c0}", tag="res")
                        nc.vector.scalar_tensor_tensor(
                            out=res[:],
                            in0=o_ps[:],
                            scalar=gR[c0][:],
                            in1=x_op,
                            op0=AluOp.mult,
                            op1=AluOp.add,
                        )
                        nc.gpsimd.dma_start(
                            out=out_b[c0 * 128:(c0 + 1) * 128,
                                      n0 * 512:(n0 + 1) * 512],
                            in_=res[:],
                        )

    nc.compile()
    return nc


def _get_nc():
    if "nc" not in _cache:
        _cache["nc"] = _build_nc()
    return _cache["nc"]


def _make_in_maps(x: np.ndarray, gamma: np.ndarray):
    x = np.ascontiguousarray(np.asarray(x, dtype=np.float32))
    gamma = np.asarray(gamma, dtype=np.float32).reshape(1, 1)
    q16 = x.reshape(B, HW, C).astype(ml_dtypes.bfloat16)
    qt16 = np.ascontiguousarray(q16.transpose(0, 2, 1))
    q016 = np.ascontiguousarray(q16[0])
    q32 = x.reshape(B, HW, C)
    in_maps = []
    for i in range(NCORES):
        idx = [i, i + NCORES]
        m = {
            "xq16": np.ascontiguousarray(q16[idx]),
            "x0q16": q016,
            "qt16": np.ascontiguousarray(qt16[idx]),
            "gamma": gamma,
        }
        if X32:
            m["xq32"] = np.ascontiguousarray(q32[idx])
        in_maps.append(m)
    return in_maps


def kernel(x: np.ndarray, gamma: np.ndarray) -> np.ndarray:
    from concourse import bass_utils

    nc = _get_nc()
    in_maps = _make_in_maps(x, gamma)
    res = bass_utils.run_bass_kernel_spmd(
        nc, in_maps, core_ids=list(range(NCORES))
    )
    outp = np.empty((B, C, HW), np.float32)
    for i in range(NCORES):
        o = res.results[i]["out"]
        outp[i] = o[0]
        outp[i + NCORES] = o[1]
    return outp.reshape(B, C, 64, 64)

